# revision 22
# baseline (speedup 1.0000x reference)
"""Trainium2 Bass kernel for nn_Classifier_22299470201420 (retrieval_knn).

Reference computation:
    hv   = (samples - 0.5) @ W.T          # [B, D] random projection
    bip  = where(hv > 0, 1, -1)           # bipolar hypervector
    dots = bip @ (2*centroids - 1).T      # [B, C] bipolar dot products
    sim  = int32(0.5 * (D + dots))        # hamming similarity counts

Sharding: data-parallel over the batch dim — each of the 8 cores gets
B/8 = 512 samples; W and centroids are replicated (no collectives).

Device kernel (per core):
  - samples-0.5 is split on the host into an fp8e4m3 hi part and a bf16
    lo residual (x == hi + lo up to ~2^-13 relative, far below the fp32
    noise floor of the reference's own accumulation). W is {-1,+1} — exact
    in both fp8 and bf16 — so hv accumulates in fp32 PSUM with fp32-level
    accuracy while the PE runs at fp8/bf16 speed.
  - hi pass: 4 DoubleRow fp8 matmuls (K=256 each); lo pass: 8 bf16
    matmuls (K=128) — all accumulating into one PSUM bank per d-tile,
    producing hv^T tiles [d=128, b=512].
  - ScalarE Sign() turns hv^T into bipolar fp8 tiles, paired per two
    d-tiles; matmul2 runs fp8 DoubleRow over the pairs, accumulating all
    79 d-tiles into one persistent PSUM bank [112, 512] of dot products.
  - D is zero-padded 10000 -> 10112 (79*128) in both W^T and centroids^T,
    so padded dims contribute exactly 0 to the dots.
  - The final affine 0.5*(D+dots) + int32 cast + transpose happens on the
    host on the tiny [100, 512] per-core outputs.
"""

import os

import numpy as np
import ml_dtypes

B, F, D, C = 4096, 1024, 10000, 100
NCORES = 8
BC = B // NCORES          # samples per core
NT = 79                   # number of 128-wide d tiles
DPAD = NT * 128           # 10112
FG = F // 128             # 8 f-chunks of 128

bf16 = ml_dtypes.bfloat16
f8 = ml_dtypes.float8_e4m3
CP = 112                  # C padded so fp8 DoubleRow weight strides are 16B-aligned

_prog_cache = {}


def _build_program(reps=1, hvp_bufs=6, bipp_bufs=3, wtp_bufs=8):
    key = ("nc", reps, hvp_bufs, bipp_bufs, wtp_bufs)
    if key in _prog_cache:
        return _prog_cache[key]

    from contextlib import ExitStack
    import concourse.bacc as bacc
    import concourse.tile as tile
    import concourse.mybir as mybir

    mbf16 = mybir.dt.bfloat16
    mf8 = mybir.dt.float8e4
    mf32 = mybir.dt.float32
    DR = mybir.MatmulPerfMode.DoubleRow

    # disable_frame_to_traceback keeps source paths out of the BIR so the
    # persistent compile cache is stable across working directories
    nc = bacc.Bacc(
        "TRN2", target_bir_lowering=False, debug=False,
        disable_frame_to_traceback=True,
    )

    st_hi_d = nc.dram_tensor("st_hi", [128, FG, BC], mf8, kind="ExternalInput")
    st_lo_d = nc.dram_tensor("st_lo", [128, FG, BC], mbf16, kind="ExternalInput")
    wt8_d = nc.dram_tensor("wt8", [NT, 128, FG, 128], mf8, kind="ExternalInput")
    wt16_d = nc.dram_tensor("wt16", [NT, 128, FG, 128], mbf16, kind="ExternalInput")
    cb_d = nc.dram_tensor("cb", [128, NT, CP], mf8, kind="ExternalInput")
    dots_d = nc.dram_tensor("dots", [C, BC], mf32, kind="ExternalOutput")

    with tile.TileContext(nc) as tc, ExitStack() as ctx:
        const = ctx.enter_context(tc.tile_pool(name="const", bufs=1))
        wtp = ctx.enter_context(tc.tile_pool(name="wtp", bufs=wtp_bufs))
        hvp = ctx.enter_context(tc.tile_pool(name="hvp", bufs=hvp_bufs, space="PSUM"))
        dotsp = ctx.enter_context(tc.tile_pool(name="dotsp", bufs=1, space="PSUM"))
        bipp = ctx.enter_context(tc.tile_pool(name="bipp", bufs=bipp_bufs))

        st_hi = const.tile([128, FG, BC], mf8, tag="st_hi")
        nc.sync.dma_start(st_hi[:], st_hi_d[:])
        st_lo = const.tile([128, FG, BC], mbf16, tag="st_lo")
        nc.sync.dma_start(st_lo[:], st_lo_d[:])
        cb = const.tile([128, NT, CP], mf8, tag="cb")
        nc.sync.dma_start(cb[:], cb_d[:])

        pd = dotsp.tile([CP, BC], mf32)

        def body():
            bip2 = None
            for dt in range(NT):
                wt8 = wtp.tile([128, FG, 128], mf8, tag="wt8")
                nc.sync.dma_start(wt8[:], wt8_d[dt])
                wt16 = wtp.tile([128, FG, 128], mbf16, tag="wt16")
                nc.sync.dma_start(wt16[:], wt16_d[dt])
                ph = hvp.tile([128, BC], mf32)
                # hi pass: fp8 DoubleRow, contracts 256 f's per matmul
                for u in range(FG // 2):
                    nc.tensor.matmul(
                        ph[:],
                        lhsT=wt8[:, 2 * u : 2 * u + 2, :],
                        rhs=st_hi[:, 2 * u : 2 * u + 2, :],
                        start=(u == 0), stop=False, perf_mode=DR,
                    )
                # lo pass: bf16, contracts 128 f's per matmul
                for g in range(FG):
                    nc.tensor.matmul(
                        ph[:],
                        lhsT=wt16[:, g, :],
                        rhs=st_lo[:, g, :],
                        start=False, stop=(g == FG - 1),
                    )
                # sign -> fp8 bipolar tile; pair consecutive d-tiles so
                # matmul2 can run fp8 DoubleRow over d-tile pairs
                if dt % 2 == 0:
                    bip2 = bipp.tile([128, 2, BC], mf8)
                nc.scalar.activation(
                    bip2[:, dt % 2, :], ph[:], mybir.ActivationFunctionType.Sign
                )
                if dt % 2 == 1:
                    nc.tensor.matmul(
                        pd[:], lhsT=cb[:, dt - 1 : dt + 1, :], rhs=bip2[:],
                        start=(dt == 1), stop=False, perf_mode=DR,
                    )
                elif dt == NT - 1:
                    # NT is odd: last d-tile is a plain fp8 matmul
                    nc.tensor.matmul(
                        pd[:], lhsT=cb[:, dt, :], rhs=bip2[:, 0, :],
                        start=False, stop=True,
                    )

        if reps == 1:
            body()
        else:
            # benchmarking only: repeat the compute in a HW loop so device
            # time can be extracted as a wall-clock differential
            with tc.For_i(0, reps, 1):
                body()

        out_sb = const.tile([C, BC], mf32, tag="out_sb")
        nc.scalar.copy(out_sb[:], pd[:C, :])
        nc.sync.dma_start(dots_d[:], out_sb[:])

    nc.compile()
    # Rewrite source-location debug info to constants so the serialized BIR
    # (and therefore the persistent compile-cache key) is independent of
    # file paths and call sites.
    def _neutral(d):
        # only OpDebugInfo carries source paths; other debug types are inert
        if d is None or not hasattr(d, "filename"):
            return d
        return type(d)(
            op_name=d.op_name, tensorizer_id=d.tensorizer_id,
            filename="kernel.py", lineno=0,
            bass_funcname=d.bass_funcname, kernel_name=d.kernel_name,
            ant_traceback=None, ant_layer=d.ant_layer,
            ant_annotation=d.ant_annotation,
        )

    for fn in nc.m.functions:
        for blk in fn.blocks:
            for inst in blk.instructions:
                if inst.debug is not None:
                    inst.debug = _neutral(inst.debug)
        for alloc in fn.allocations:
            for ml in getattr(alloc, "memorylocations", None) or []:
                if getattr(ml, "ant_debug", None) is not None:
                    ml.ant_debug = _neutral(ml.ant_debug)
    _prog_cache[key] = nc
    return nc


def _pack_w(W, dtype):
    # W^T padded [F, DPAD], packed so each d-tile is one contiguous
    # [128, FG*128] SBUF image: packed[dt, p, g, j] = W^T[g*128+p, dt*128+j]
    WT = np.zeros((F, DPAD), dtype=dtype)
    WT[:, :D] = W.astype(dtype).T
    return np.ascontiguousarray(
        WT.reshape(FG, 128, NT, 128).transpose(2, 1, 0, 3)
    )


def _pack_cb(centroids):
    # centroids^T (bipolar) padded [DPAD, CP]: packed[p, t, c] = cb^T[t*128+p, c]
    cbT = np.zeros((DPAD, CP), dtype=f8)
    cbT[:D, :C] = (2.0 * centroids.astype(np.float32) - 1.0).astype(f8).T
    return np.ascontiguousarray(cbT.reshape(NT, 128, CP).transpose(1, 0, 2))


def _pack_st(part_c):
    # part_c: [BC, F] -> packed[p, g, b] = part_c.T[g*128+p, b]
    return np.ascontiguousarray(part_c.T.reshape(FG, 128, BC).transpose(1, 0, 2))


def _enable_jax_compile_cache():
    # Persistent XLA/NEFF compile cache so repeated invocations (fresh
    # processes included) skip the multi-minute neuronx-cc compile.
    try:
        import jax

        d = os.path.expanduser("~/.cache/trn_knn_kernel_jax_cache")
        os.makedirs(d, exist_ok=True)
        jax.config.update("jax_compilation_cache_dir", d)
        jax.config.update("jax_persistent_cache_min_entry_size_bytes", 0)
        jax.config.update("jax_persistent_cache_min_compile_time_secs", 0)
    except Exception:
        pass


def _run(inputs, trace=False, reps=1):
    _enable_jax_compile_cache()
    from concourse.bass_utils import run_bass_kernel_spmd

    samples = np.asarray(inputs["samples"], dtype=np.float32)
    W = np.asarray(inputs["W"], dtype=np.float32)
    centroids = np.asarray(inputs["centroids"], dtype=np.float32)
    assert samples.shape == (B, F) and W.shape == (D, F) and centroids.shape == (C, D)

    x = samples - 0.5
    hi = x.astype(f8)
    lo = (x - hi.astype(np.float32)).astype(bf16)
    wt8_packed = _pack_w(W, f8)
    wt16_packed = _pack_w(W, bf16)
    cb_packed = _pack_cb(centroids)

    in_maps = []
    for i in range(NCORES):
        sl = slice(i * BC, (i + 1) * BC)
        in_maps.append(
            {
                "st_hi": _pack_st(hi[sl]),
                "st_lo": _pack_st(lo[sl]),
                "wt8": wt8_packed,
                "wt16": wt16_packed,
                "cb": cb_packed,
            }
        )

    nc = _build_program(reps=reps)
    res = run_bass_kernel_spmd(nc, in_maps, list(range(NCORES)), trace=trace)

    out = np.empty((B, C), dtype=np.int32)
    for i in range(NCORES):
        dots = np.asarray(res.results[i]["dots"], dtype=np.float32)  # [C, BC]
        sim = np.rint(0.5 * (np.float64(D) + dots.astype(np.float64)))
        out[i * BC : (i + 1) * BC, :] = sim.T.astype(np.int32)
    return out, res


def kernel(samples, W, centroids):
    out, _ = _run({"samples": samples, "W": W, "centroids": centroids})
    return out
